# revision 14
# baseline (speedup 1.0000x reference)
"""Trainium2 Bass kernel for nn_CausalFieldLayer.

Strategy (validated on host):
  * h = x@W_in is only consumed by three 1024->16 projections, so W_in folds
    into a single [1024,48] matrix W_all (plus a ones column for sum_d x).
  * The complex-octonion associator Jv is a fixed trilinear form; each cmul is
    computed as outer-product expansion (PE matmuls with 0/1 matrices),
    an elementwise multiply (DVE), and a contraction by G2 [256,16] (PE).
  * Everything downstream of Jv is linear: J_expand/antisym/Pi_source/
    Pi_target/spinor-trace/W_out/alpha fold into P1,P2 [16,1024] on host.
  * The FFT conv is a 64-tap causal conv -> Toeplitz matmul on PE.
  * Layernorm mean is linear -> folded into the final matmul as a -mu
    stationary row; variance via ScalarE Square+accum; normalize on DVE.
  * Data-parallel over B=8: core i handles batch element i.

Channel-major tensors use 32-aligned partition groups because matmul operand
base partitions must be in {0,32,64} and psum->sbuf copies cannot shift
partitions:
  pall psum/sbuf [80,Tc]: ps@0-15, sumx@16, pl@32-47, pa@64-79
  JJ [66,T]: Jv@0-15, Jc@32-47, -mu@64, ones@65 (zero pcat rows elsewhere)

The middle path runs in bf16 (validated: end-to-end error ~8e-4 relative,
because |out| ~ 0.07 * |x| so associator-path errors are damped 14x).
Residual add and normalization stay fp32.
"""

from contextlib import ExitStack

import numpy as np
import ml_dtypes

import concourse.bass as bass
import concourse.bacc as bacc
import concourse.mybir as mybir
import concourse.tile as tile
from concourse.bass_utils import run_bass_kernel_spmd

BF = ml_dtypes.bfloat16
F32 = np.float32

B, N, DM = 8, 4096, 1024
NCORES = 8
KSIZE = 64

EPS = 1e-5


# ----------------------------------------------------------------------------
# Host-side folding
# ----------------------------------------------------------------------------

def fold_params(inp):
    f64 = np.float64
    f = np.asarray(inp["oct_struct"], f64)  # [8,8,8] f[j,k,i]
    W_cat = np.concatenate(
        [np.asarray(inp[k], f64) for k in ("W_sigma", "W_lam", "W_alp")], axis=1
    )  # [1024,48]
    W_all = np.asarray(inp["W_in"], f64) @ W_cat
    b_all = np.asarray(inp["b_in"], f64) @ W_cat + np.concatenate(
        [np.asarray(inp[k], f64) for k in ("b_sigma", "b_lam", "b_alp")]
    )

    # cmul structure tensor G[i,j,k]: cmul(u,v)_i = sum_jk G[i,j,k] u_j v_k
    G = np.zeros((16, 16, 16), f64)
    ft = np.transpose(f, (2, 0, 1))  # ft[i,j,k] = f[j,k,i]
    G[:8, :8, :8] = ft
    G[:8, 8:, 8:] = -ft
    G[8:, :8, 8:] = ft
    G[8:, 8:, :8] = ft
    G2 = G.transpose(1, 2, 0).reshape(256, 16)  # [jk, i]

    JE = np.asarray(inp["J_expand"], f64)
    A = (JE - np.transpose(JE, (0, 2, 1))).reshape(16, 256)

    Gamma = np.einsum("ab,bcd->cd", np.asarray(inp["tetrad"], f64),
                      np.asarray(inp["gammas"], f64))
    sp = np.einsum("gdk,gd->k", np.asarray(inp["Pi_spinor"], f64), Gamma)

    PiS = np.asarray(inp["Pi_source"], f64).reshape(256, 16)
    PiT = np.asarray(inp["Pi_target"], f64).reshape(256, 16)
    C = (A @ PiS) @ PiT.T * np.tile(sp, 16)[None, :]

    kw = np.asarray(inp["kweights"], f64)
    alpha = kw[0]
    W_out = np.asarray(inp["W_out"], f64)
    P1 = alpha * (A @ W_out)
    P2 = (1.0 - alpha) * (C @ W_out)
    b_out = np.asarray(inp["b_out"], f64)

    # wcat [1024, 52]: cols 0-15 Wps, 16 ones(sumx), 17-32 Wpl, 33-48 Wpa
    wcat = np.zeros((DM, 52), f64)
    wcat[:, 0:16] = W_all[:, 0:16]
    wcat[:, 16] = 1.0
    wcat[:, 17:33] = W_all[:, 16:32]
    wcat[:, 33:49] = W_all[:, 32:48]

    # per-partition bias for the pall evac [80,1]
    ball = np.zeros((80, 1), f64)
    ball[0:16, 0] = b_all[0:16]
    ball[32:48, 0] = b_all[16:32]
    ball[64:80, 0] = b_all[32:48]

    # pcat [66, 1024]: 0-15 P1 (Jv), 32-47 P2 (Jc), 64 ones (-mu), 65 b_out
    pcat = np.zeros((66, DM), f64)
    pcat[0:16] = P1
    pcat[32:48] = P2
    pcat[64] = 1.0
    pcat[65] = b_out

    # svec [48,1]: row-sums of P1 at 0-15, of P2 at 32-47 (for sum_d out)
    svec = np.zeros((48, 1), f64)
    svec[0:16, 0] = P1.sum(axis=1)
    svec[32:48, 0] = P2.sum(axis=1)
    sumb = float(b_out.sum())

    # sel17 [17,1]: selects the sumx row (16) of pall
    sel17 = np.zeros((17, 1), f64)
    sel17[16, 0] = 1.0

    # expansion matrices, replicated at partition bases 0/32/64
    rrep = np.zeros((16, 256), f64)
    rtile = np.zeros((16, 256), f64)
    for j in range(16):
        for k in range(16):
            rrep[j, j * 16 + k] = 1.0
            rtile[k, j * 16 + k] = 1.0
    rrep3 = np.zeros((128, 256), f64)
    rtile3 = np.zeros((128, 256), f64)
    for base in (0, 32, 64):
        rrep3[base:base + 16] = rrep
        rtile3[base:base + 16] = rtile

    # G2 chunks: [128, 4, 16] = [G2a, G2b, -G2a, -G2b]
    g2c = np.zeros((128, 4, 16), f64)
    g2c[:, 0] = G2[:128]
    g2c[:, 1] = G2[128:]
    g2c[:, 2] = -G2[:128]
    g2c[:, 3] = -G2[128:]

    # conv Toeplitz [192,128]: out[tl] = sum_sl afull[sl, tl] * Jv[t0-64+sl]
    afull = np.zeros((192, 128), f64)
    for sl in range(192):
        for tl in range(128):
            tap = tl + 64 - sl
            if 0 <= tap < KSIZE:
                afull[sl, tl] = kw[tap]
    a1p = np.zeros((128, 128), f64)
    a1p[64:128] = afull[0:64]  # stored at partition base 64

    ln_g = np.asarray(inp["ln_g"], f64)
    ln_b = np.asarray(inp["ln_b"], f64)

    return dict(
        wcat=wcat.astype(BF),
        ball=ball.astype(F32),
        rrep3=rrep3.astype(BF),
        rtile3=rtile3.astype(BF),
        g2c=g2c.astype(BF),
        a1p=a1p.astype(BF),
        a2=afull[64:].astype(BF),
        pcat=pcat.astype(BF),
        svec=svec.astype(BF),
        sel17=sel17.astype(BF),
        sumb=sumb,
        ident=np.eye(128).astype(BF),
        ln_g=ln_g.astype(F32),
        ln_b=ln_b.astype(F32),
        g_trivial=bool(np.all(ln_g == 1.0)),
        b_trivial=bool(np.all(ln_b == 0.0)),
    )


# ----------------------------------------------------------------------------
# Device kernel
# ----------------------------------------------------------------------------

def build_kernel(nc, T, sumb, g_trivial, b_trivial, reps=1):
    dt = mybir.dt
    P = 128
    TC = 512                 # token chunk
    TPC = TC // P            # token tiles per chunk (4)
    NCH = T // TC            # chunks
    KT = T // P              # token tiles total

    x_d = nc.declare_dram_parameter("x", [T, DM], dt.float32, isOutput=False)
    y_d = nc.declare_dram_parameter("y", [T, DM], dt.float32, isOutput=True)
    wcat_d = nc.declare_dram_parameter("wcat", [DM, 52], dt.bfloat16, isOutput=False)
    ball_d = nc.declare_dram_parameter("ball", [80, 1], dt.float32, isOutput=False)
    rrep_d = nc.declare_dram_parameter("rrep3", [128, 256], dt.bfloat16, isOutput=False)
    rtile_d = nc.declare_dram_parameter("rtile3", [128, 256], dt.bfloat16, isOutput=False)
    g2c_d = nc.declare_dram_parameter("g2c", [128, 4, 16], dt.bfloat16, isOutput=False)
    a1p_d = nc.declare_dram_parameter("a1p", [128, 128], dt.bfloat16, isOutput=False)
    a2_d = nc.declare_dram_parameter("a2", [128, 128], dt.bfloat16, isOutput=False)
    pcat_d = nc.declare_dram_parameter("pcat", [66, DM], dt.bfloat16, isOutput=False)
    svec_d = nc.declare_dram_parameter("svec", [48, 1], dt.bfloat16, isOutput=False)
    sel17_d = nc.declare_dram_parameter("sel17", [17, 1], dt.bfloat16, isOutput=False)
    ident_d = nc.declare_dram_parameter("ident", [128, 128], dt.bfloat16, isOutput=False)
    lng_d = nc.declare_dram_parameter("lng", [DM], dt.float32, isOutput=False)
    lnb_d = nc.declare_dram_parameter("lnb", [DM], dt.float32, isOutput=False)

    with tile.TileContext(nc) as tc, ExitStack() as ctx:
        consts = ctx.enter_context(tc.tile_pool(name="consts", bufs=1))
        persist = ctx.enter_context(tc.tile_pool(name="persist", bufs=1))
        xin = ctx.enter_context(tc.tile_pool(name="xin", bufs=2))
        xbp = ctx.enter_context(tc.tile_pool(name="xbp", bufs=2))
        xtp = ctx.enter_context(tc.tile_pool(name="xtp", bufs=2))
        mid = ctx.enter_context(tc.tile_pool(name="mid", bufs=2))
        ycp = ctx.enter_context(tc.tile_pool(name="ycp", bufs=2))
        stat = ctx.enter_context(tc.tile_pool(name="stat", bufs=2))
        psA = ctx.enter_context(tc.tile_pool(name="psA", bufs=3, space="PSUM"))
        psS = ctx.enter_context(tc.tile_pool(name="psS", bufs=3, space="PSUM"))
        psO = ctx.enter_context(tc.tile_pool(name="psO", bufs=2, space="PSUM"))

        # ---- constants into SBUF ----
        wcat_sb = consts.tile([P, 8, 52], dt.bfloat16)
        nc.sync.dma_start(wcat_sb[:], wcat_d.rearrange("(a p) m -> p a m", p=P))
        ball_sb = consts.tile([80, 1], dt.float32)
        nc.sync.dma_start(ball_sb[:], ball_d[:])
        rrep_sb = consts.tile([128, 256], dt.bfloat16)
        nc.sync.dma_start(rrep_sb[:], rrep_d[:])
        rtile_sb = consts.tile([128, 256], dt.bfloat16)
        nc.sync.dma_start(rtile_sb[:], rtile_d[:])
        g2_sb = consts.tile([128, 4, 16], dt.bfloat16)
        nc.sync.dma_start(g2_sb[:], g2c_d[:])
        a1p_sb = consts.tile([128, 128], dt.bfloat16)
        nc.sync.dma_start(a1p_sb[:], a1p_d[:])
        a2_sb = consts.tile([128, 128], dt.bfloat16)
        nc.sync.dma_start(a2_sb[:], a2_d[:])
        pcat_sb = consts.tile([66, DM], dt.bfloat16)
        nc.sync.dma_start(pcat_sb[:], pcat_d[:])
        svec_sb = consts.tile([48, 1], dt.bfloat16)
        nc.sync.dma_start(svec_sb[:], svec_d[:])
        sel17_sb = consts.tile([17, 1], dt.bfloat16)
        nc.sync.dma_start(sel17_sb[:], sel17_d[:])
        ident_sb = consts.tile([128, 128], dt.bfloat16)
        nc.sync.dma_start(ident_sb[:], ident_d[:])
        eps_sb = consts.tile([P, 1], dt.float32)
        nc.vector.memset(eps_sb[:], EPS)

        gb_sb = None
        if not (g_trivial and b_trivial):
            gb_sb = consts.tile([P, 2, DM], dt.float32)
            nc.sync.dma_start(gb_sb[:, 0, :], lng_d[None, :].to_broadcast((P, DM)))
            nc.sync.dma_start(gb_sb[:, 1, :], lnb_d[None, :].to_broadcast((P, DM)))

        # ---- persistent ----
        rep_cm = tc.For_i(0, reps, 1) if reps > 1 else None
        if rep_cm is not None:
            rep_cm.__enter__()
        JJ = persist.tile([66, T], dt.bfloat16)       # Jv/Jc/-mu/ones
        # zero the unused gap rows (16-31, 48-63) so full-range reads are valid
        nc.vector.memset(JJ[:], 0.0)
        # rows 64 (-mu, rewritten per chunk) and 65 (ones) — base must be 64
        nc.vector.memset(JJ[64:66, :], 1.0)
        JvT = persist.tile([P, KT, 16], dt.bfloat16)  # token-major Jv

        def cmul_w(u_ap, u_base, v_ap, v_base, w_sb, vtile_sb, reuse_vtile):
            """w[:,h,:] = (expand_rep u) * (expand_tile v) for h in 0,1."""
            for h in range(2):
                prep = psA.tile([P, TC], dt.float32, tag="psA")
                nc.tensor.matmul(
                    prep[:],
                    rrep_sb[u_base:u_base + 16, bass.ts(h, 128)],
                    u_ap,
                )
                if not reuse_vtile:
                    ptile = psA.tile([P, TC], dt.float32, tag="psA")
                    nc.tensor.matmul(
                        ptile[:],
                        rtile_sb[v_base:v_base + 16, bass.ts(h, 128)],
                        v_ap,
                    )
                    nc.scalar.copy(vtile_sb[:, h, :], ptile[:])
                nc.vector.tensor_mul(w_sb[:, h, :], prep[:], vtile_sb[:, h, :])

        def contract(out_ps, pairs, tile_position=None):
            n = len(pairs)
            for i, (gi, w_sb_i, h) in enumerate(pairs):
                nc.tensor.matmul(
                    out_ps, g2_sb[:, gi, :], w_sb_i[:, h, :],
                    start=(i == 0), stop=(i == n - 1),
                )

        for c in range(NCH):
            t0 = c * TC
            csl = slice(t0, t0 + TC)

            # ---- load x chunk ----
            x32 = xin.tile([P, TPC, DM], dt.float32)
            nc.sync.dma_start(
                x32[:], x_d[csl, :].rearrange("(j p) d -> p j d", p=P)
            )
            xb = xbp.tile([P, TPC, DM], dt.bfloat16)
            nc.vector.tensor_copy(xb[:], x32[:])

            # ---- transpose x (PE) -> xT [128, 8, TC] bf16 ----
            xT = xtp.tile([P, 8, TC], dt.bfloat16)
            for j in range(TPC):
                pxT = psA.tile([P, 8, 128], dt.bfloat16, tag="psA")
                for a in range(8):
                    nc.tensor.transpose(
                        pxT[:, a, :], xb[:, j, bass.ts(a, 128)], ident_sb[:]
                    )
                nc.scalar.copy(xT[:, :, bass.ts(j, 128)], pxT[:])

            # ---- projections (col-tiled): ps@0-15, sumx@16, pl@32, pa@64 ----
            pps = psS.tile([80, TC], dt.float32, tag="psS")
            groups = [(0, 0, 17), (32, 17, 16), (64, 33, 16)]
            for gi, (obase, wofs, m) in enumerate(groups):
                for a in range(8):
                    nc.tensor.matmul(
                        pps[obase:obase + m, :],
                        wcat_sb[:, a, wofs:wofs + m],
                        xT[:, a, :],
                        start=(a == 0), stop=(a == 7),
                        tile_position=(0, obase),
                    )
            pall = mid.tile([80, TC], dt.bfloat16, tag="pall")
            for lo, hi in ((0, 17), (32, 48), (64, 80)):
                nc.vector.tensor_scalar(
                    pall[lo:hi, :], pps[lo:hi, :], ball_sb[lo:hi, :], None,
                    mybir.AluOpType.add,
                )

            # ---- associator ----
            w1 = mid.tile([P, 2, TC], dt.bfloat16, tag="w1")
            w2 = mid.tile([P, 2, TC], dt.bfloat16, tag="w2")
            vt_pl = mid.tile([P, 2, TC], dt.bfloat16, tag="vtpl")
            vt_pa = mid.tile([P, 2, TC], dt.bfloat16, tag="vtpa")
            ps_ap = pall[0:16, :]
            pl_ap = pall[32:48, :]
            pa_ap = pall[64:80, :]
            cmul_w(ps_ap, 0, pl_ap, 32, w1, vt_pl, False)
            cmul_w(pl_ap, 32, pa_ap, 64, w2, vt_pa, False)

            pU = psS.tile([16, TC], dt.float32, tag="psS")
            contract(pU[:], [(0, w1, 0), (1, w1, 1)])
            pY = psS.tile([16, TC], dt.float32, tag="psS")
            contract(pY[:], [(0, w2, 0), (1, w2, 1)])
            U_sb = mid.tile([16, TC], dt.bfloat16, tag="U")
            nc.scalar.copy(U_sb[:], pU[:])
            Y_sb = mid.tile([16, TC], dt.bfloat16, tag="Y")
            nc.scalar.copy(Y_sb[:], pY[:])

            w3 = mid.tile([P, 2, TC], dt.bfloat16, tag="w3")
            w4 = mid.tile([P, 2, TC], dt.bfloat16, tag="w4")
            vt_y = mid.tile([P, 2, TC], dt.bfloat16, tag="vty")
            cmul_w(U_sb[:], 0, None, 0, w3, vt_pa, True)   # U_rep * pa_tile
            cmul_w(ps_ap, 0, Y_sb[:], 0, w4, vt_y, False)  # ps_rep * Y_tile

            pJv = psS.tile([16, TC], dt.float32, tag="psS")
            contract(pJv[:], [(0, w3, 0), (1, w3, 1), (2, w4, 0), (3, w4, 1)])
            nc.scalar.copy(JJ[0:16, csl], pJv[:])

            # ---- Jv token-major (for conv stationary) ----
            pjvT = psS.tile([P, TPC, 16], dt.bfloat16, tag="psS")
            for j in range(TPC):
                nc.tensor.transpose(
                    pjvT[:, j, :],
                    JJ[0:16, t0 + j * P:t0 + (j + 1) * P],
                    ident_sb[0:16, 0:16],
                )
            nc.scalar.copy(JvT[:, c * TPC:(c + 1) * TPC, :], pjvT[:])

            # ---- causal conv (Toeplitz matmuls) -> Jc at rows 32-47 ----
            pJc = psS.tile([48, TC], dt.float32, tag="psS")
            for j in range(TPC):
                g = c * TPC + j
                osl = pJc[32:48, bass.ts(j, 128)]
                if g > 0:
                    nc.tensor.matmul(
                        osl, JvT[64:128, g - 1, :], a1p_sb[64:128, :],
                        start=True, stop=False, tile_position=(64, 32),
                    )
                    nc.tensor.matmul(
                        osl, JvT[:, g, :], a2_sb[:],
                        start=False, stop=True, tile_position=(0, 32),
                    )
                else:
                    nc.tensor.matmul(
                        osl, JvT[:, g, :], a2_sb[:],
                        start=True, stop=True, tile_position=(0, 32),
                    )
            nc.scalar.copy(JJ[32:48, csl], pJc[32:48, :])

            # ---- mean: -mu = -(sum_d x + sum_d out + sum(b_out))/DM ----
            pmu = psS.tile([65, TC], dt.float32, tag="psS")
            nc.tensor.matmul(
                pmu[64:65, :], svec_sb[:], JJ[0:48, csl],
                start=True, stop=False, tile_position=(0, 64),
            )
            nc.tensor.matmul(
                pmu[64:65, :], sel17_sb[:], pall[0:17, :],
                start=False, stop=True, tile_position=(0, 64),
            )
            nc.scalar.activation(
                JJ[64:65, csl], pmu[64:65, :],
                mybir.ActivationFunctionType.Copy,
                bias=-sumb / DM, scale=-1.0 / DM,
            )

            # ---- final matmul + residual + layernorm ----
            yc = ycp.tile([P, TPC, DM], dt.float32)
            ssq = stat.tile([P, TPC], dt.float32, tag="ssq")
            sqs = stat.tile([P, DM], dt.bfloat16, tag="sqs")
            for j in range(TPC):
                tsl = slice(t0 + j * P, t0 + (j + 1) * P)
                for nh in range(2):
                    pout = psO.tile([P, 512], dt.float32, tag="psO")
                    nc.tensor.matmul(
                        pout[:], JJ[0:66, tsl], pcat_sb[:, bass.ts(nh, 512)]
                    )
                    nc.vector.tensor_add(
                        yc[:, j, bass.ts(nh, 512)],
                        x32[:, j, bass.ts(nh, 512)],
                        pout[:],
                    )
                nc.scalar.activation(
                    sqs[:], yc[:, j, :], mybir.ActivationFunctionType.Square,
                    accum_out=ssq[:, j:j + 1],
                )

            var4 = stat.tile([P, TPC], dt.float32, tag="var4")
            nc.scalar.activation(
                var4[:], ssq[:], mybir.ActivationFunctionType.Copy,
                bias=0.0, scale=1.0 / DM,
            )
            sig4 = stat.tile([P, TPC], dt.float32, tag="sig4")
            nc.scalar.activation(
                sig4[:], var4[:], mybir.ActivationFunctionType.Sqrt, bias=eps_sb[:]
            )
            rsig4 = stat.tile([P, TPC], dt.float32, tag="rsig4")
            nc.vector.reciprocal(rsig4[:], sig4[:])

            for j in range(TPC):
                nc.vector.tensor_scalar_mul(
                    yc[:, j, :], yc[:, j, :], rsig4[:, j:j + 1]
                )
                if gb_sb is not None:
                    nc.vector.tensor_mul(yc[:, j, :], yc[:, j, :], gb_sb[:, 0, :])
                    nc.vector.tensor_add(yc[:, j, :], yc[:, j, :], gb_sb[:, 1, :])

            nc.sync.dma_start(
                y_d[csl, :].rearrange("(j p) d -> p j d", p=P), yc[:]
            )

        if rep_cm is not None:
            rep_cm.__exit__(None, None, None)

    return nc


# ----------------------------------------------------------------------------
# Entry point
# ----------------------------------------------------------------------------

def _const_map(fp):
    return {
        "wcat": fp["wcat"], "ball": fp["ball"], "rrep3": fp["rrep3"],
        "rtile3": fp["rtile3"], "g2c": fp["g2c"], "a1p": fp["a1p"],
        "a2": fp["a2"], "pcat": fp["pcat"], "svec": fp["svec"],
        "sel17": fp["sel17"], "ident": fp["ident"],
        "lng": fp["ln_g"], "lnb": fp["ln_b"],
    }


def _run(inputs, trace=False):
    x = inputs["x"]
    assert x.shape == (B, N, DM), x.shape
    fp = fold_params(inputs)

    nc = bacc.Bacc("TRN2", target_bir_lowering=False)
    build_kernel(nc, N, fp["sumb"], fp["g_trivial"], fp["b_trivial"])
    nc.finalize()

    cm = _const_map(fp)
    in_maps = [
        {"x": np.ascontiguousarray(x[i], dtype=F32), **cm} for i in range(NCORES)
    ]
    return run_bass_kernel_spmd(nc, in_maps, list(range(NCORES)), trace=trace)


def kernel(**inputs):
    inputs = {k: np.asarray(v) for k, v in inputs.items()}
    res = _run(inputs)
    y = np.stack([res.results[i]["y"] for i in range(NCORES)], axis=0)
    return y.astype(np.float32)


def timed_run(inputs):
    """Run with NTFF profiling; returns HW exec time in ns (or None)."""
    res = _run({k: np.asarray(v) for k, v in inputs.items()}, trace=True)
    return res.exec_time_ns


if __name__ == "__main__":
    import reference

    inp = reference.setup_inputs()
    out = kernel(**{k: np.asarray(v) for k, v in inp.items()})
    print("kernel output", out.shape, out.dtype)
